# revision 16
# baseline (speedup 1.0000x reference)
"""Trainium2 Bass kernel for nn_AbilityGammaAttention.

Reference computation (per batch b):
    ws = s_j @ Ws_w.T + Ws_b                      # (P, A)
    uh = exp_tokens @ U_w.T                       # (Q, LE, A)
    e[q,p,t] = v . tanh(uh[q,t,:] + ws[p,:])      # (Q, P, LE)
    e masked by exp_mask (tokens), joint softmax over (Q, LE) per (b, p)
    out[q,p,:] = sum_t a[q,p,t] * exp_tokens[q,t,:], zeroed where req_mask[p]==0

Sharding: data-parallel over B across the 8 NeuronCores (batch b -> core b).

Key idea (replaces the per-p tanh loop of the previous version): expand the
shifted-tanh family in a fixed basis,

    tanh(u + w) ~= c0(w) + clin(w)*u + sum_r c_r(w) * tanh(u + b_r),

with R=12 fixed shifts b_r. The c*(w) coefficient functions are least-squares
fits (precomputed on a w-grid at import; Gaussian-weighted in u with a uniform
floor for tail control). Since ws = s_j@Ws_w.T + Ws_b is host-computable, the
host evaluates the coefficients at the actual w values and uploads, per core,
stationary matrices S_r[a,p] = v_a * c_r(ws[p,a]). On device:

    e[p,t] = sum_r (S_r^T @ tanh(uhT + b_r))[p,t] + (S_lin^T @ uhT)[p,t] + mask

i.e. R ScalarE tanh passes over [A, T] + (R+2) PE matmuls, instead of P=32
tanh passes. The c0 term is a per-p constant -> cancels in the softmax over t
(the denominators come from the Exp pass's accum_out and the normalization +
req_mask is applied on the host).

Other structure:
  - Host-side token compaction with a per-rank profile: each batch's queries
    are sorted by unmasked-token count; slot i is padded to the max count at
    rank i across batches (shared static shape, ~18% fewer tokens than
    uniform padding). Padding slots are masked via an additive (m-1)*1e9
    rank-1 matmul, exactly like the reference -1e9 masking.
  - All heavy dataflow in bf16: x is uploaded bf16 and XBAR-transpose-DMA'd
    into xT (no on-device transposes for uh), uh/tanh/coefficients/apply all
    bf16 (f32 PSUM accumulation).
  - Output: per-slot apply matmuls pack 3 slots per PSUM bank (base
    partitions 0/32/64), evacuated bf16 and DMA'd out unnormalized; the host
    divides by the denominators and applies req_mask.
"""

import sys

if "/opt/trn_rl_repo" not in sys.path:
    sys.path.insert(0, "/opt/trn_rl_repo")

import numpy as np
import ml_dtypes

import concourse.bacc as bacc
import concourse.mybir as mybir
from concourse.masks import make_identity
from concourse.tile import TileContext

F32 = mybir.dt.float32
F32R = mybir.dt.float32r
BF16 = mybir.dt.bfloat16
AF = mybir.ActivationFunctionType

B, Q, LE, D, P, A = 8, 32, 128, 512, 32, 128
N_CORES = 8
DC = D // 128
R = 12                       # tanh basis size
DEBUG_DUMP = False
STAGE = 3
import os as _os
KSKIP = set(_os.environ.get('KSKIP','').split(','))  # debug: 0=uh only, 1=+tanh, 2=+e/exp, 3=full
SEG_CAP = 1536               # max tokens per segment (3 PSUM banks of f32)

# ---------------------------------------------------------------------------
# basis fit (data-independent; computed once at import)
# ---------------------------------------------------------------------------


def _build_fit(r=R, ridge=1e-6):
    b_r = np.linspace(-4.4, 4.4, r)
    u = np.linspace(-6.5, 6.5, 1301)
    rho = np.exp(-0.5 * u * u) + 0.01
    rho /= rho.sum()
    Phi = np.concatenate(
        [np.ones_like(u)[:, None], u[:, None], np.tanh(u[:, None] + b_r[None, :])],
        axis=1)
    M = Phi.T @ (rho[:, None] * Phi) + ridge * np.eye(r + 2)
    w_grid = np.linspace(-5.0, 5.0, 2001)
    G = np.tanh(u[:, None] + w_grid[None, :])
    C_grid = np.linalg.solve(M, Phi.T @ (rho[:, None] * G))   # [(r+2), Nw]
    return b_r, w_grid, C_grid


_B_R, _W_GRID, _C_GRID = _build_fit()


def _coef_eval(w):
    """Evaluate coefficient functions (rows 1..R+1: linear + R basis) at w."""
    wc = np.clip(w, _W_GRID[0], _W_GRID[-1])
    out = np.empty((R + 1,) + w.shape, dtype=np.float64)
    for i in range(R + 1):
        out[i] = np.interp(wc, _W_GRID, _C_GRID[i + 1])
    return out


def _ceil(x, m):
    return -(-x // m) * m


# ---------------------------------------------------------------------------
# device kernel
# ---------------------------------------------------------------------------


def build_kernel(layout, bound):
    """layout: tuple of segments, each a tuple of slot widths (le_i, padded so
    each segment total is a multiple of 64 -- pad carried by the last slot's
    mask only; slot widths themselves are the DMA/apply sizes)."""
    segs = [list(s) for s in layout]
    seg_T = [sum(s) for s in segs]
    T = sum(seg_T)
    seg_off = np.concatenate([[0], np.cumsum(seg_T)]).astype(int)
    assert all(t % 64 == 0 and t <= SEG_CAP for t in seg_T)
    nseg = len(segs)

    # global slot offsets/widths (in token axis), slot index = (seg, j)
    slot_off = []
    slot_w = []
    for si, s in enumerate(segs):
        o = int(seg_off[si])
        for wdt in s:
            slot_off.append(o)
            slot_w.append(int(wdt))
            o += int(wdt)
    nslots = len(slot_off)
    assert nslots == Q

    nc = bacc.Bacc("TRN2", target_bir_lowering=False, debug=False)

    x_dram = nc.dram_tensor("x", [T, D], BF16, kind="ExternalInput")
    xp_dram = nc.dram_tensor("x_pad", [128, Q * D], BF16, kind="ExternalInput")
    m_dram = nc.dram_tensor("m_row_in", [1, T], BF16, kind="ExternalInput")
    uwt_dram = nc.dram_tensor("uwT", [128, DC * A], BF16, kind="ExternalInput")
    co_dram = nc.dram_tensor("coefs", [A, (R + 1) * P], BF16, kind="ExternalInput")
    out_dram = nc.dram_tensor("out", [Q, P, D], BF16, kind="ExternalOutput")
    den_dram = nc.dram_tensor("den", [P, nseg], F32, kind="ExternalOutput")
    if DEBUG_DUMP:
        uh_dbg = nc.dram_tensor("uh_dbg", [A, T], BF16, kind="ExternalOutput")
        ef_dbg = nc.dram_tensor("ef_dbg", [P, T], BF16, kind="ExternalOutput")

    def chunks(lo, hi, step=512):
        return [(o, min(step, hi - o)) for o in range(lo, hi, step)]

    with TileContext(nc) as tc:
        with tc.tile_pool(name="live", bufs=1) as L:
            ident = L.tile([128, 128], F32)
            ident_b = L.tile([P, P], BF16)
            x_all = L.tile([128, Q * D], BF16)
            xT = L.tile([128, DC * T], BF16)
            uhT = L.tile([A, T], BF16)
            uwt_sb = L.tile([128, DC * A], BF16)
            co_sb = L.tile([A, (R + 1) * P], BF16)
            m_row = L.tile([1, T], BF16)
            ones_b = L.tile([1, P], BF16)
            e_full = L.tile([P, T], BF16)
            aT_all = L.tile([128, Q * P], BF16)
            sumh = L.tile([P, nseg], F32)
            bvals = L.tile([A, R], F32)
            nbnd = L.tile([P, 1], F32)

            make_identity(nc, ident)
            nc.vector.tensor_copy(ident_b[:], ident[0:P, 0:P])
            nc.gpsimd.memset(ones_b[:], 1.0)
            for r in range(R):
                nc.gpsimd.memset(bvals[:, r:r + 1], float(_B_R[r]))
            nc.gpsimd.memset(nbnd[:], -float(bound))

            # ---- input DMAs (xT first: uh is the critical path) ---------
            xblks = []
            for si in range(nseg):
                s0, s1 = int(seg_off[si]), int(seg_off[si + 1])
                step = _ceil((s1 - s0) // 2, 16) if s1 - s0 > 768 else (s1 - s0)
                xblks += chunks(s0, s1, step)
            for rb0, rbw in xblks:
                for c in range(DC):
                    nc.sync.dma_start(
                        xT[:, c * T + rb0:c * T + rb0 + rbw],
                        x_dram.ap()[rb0:rb0 + rbw, c * 128:(c + 1) * 128],
                        transpose=True)
            nc.scalar.dma_start(uwt_sb[:], uwt_dram[:])
            if 'co' not in KSKIP:
                nc.scalar.dma_start(co_sb[:], co_dram[:])
            if 'm' not in KSKIP:
                nc.scalar.dma_start(m_row[:], m_dram[:])
            if 'xall' not in KSKIP:
                nc.sync.dma_start(x_all[:], xp_dram[:])

            with (
                tc.tile_pool(name="scp", bufs=3) as SC,
                tc.tile_pool(name="osp", bufs=3) as OSB,
                tc.tile_pool(name="ups", bufs=1, space="PSUM") as UPS,
                tc.tile_pool(name="pse", bufs=1, space="PSUM") as PE_,
                tc.tile_pool(name="pst", bufs=2, space="PSUM") as PT,
                tc.tile_pool(name="pso", bufs=2, space="PSUM") as PO,
            ):
                # ---- uh for all tokens (segment-aligned chunks) ---------
                uh_chunks = []
                for si in range(nseg):
                    uh_chunks += chunks(int(seg_off[si]), int(seg_off[si + 1]))
                for off, w in uh_chunks:
                    ups = UPS.tile([A, 512], F32, tag="ups")
                    for c in range(DC):
                        nc.tensor.matmul(
                            ups[:, 0:w], uwt_sb[:, c * A:(c + 1) * A],
                            xT[:, c * T + off:c * T + off + w],
                            start=(c == 0), stop=(c == DC - 1))
                    nc.vector.tensor_copy(uhT[:, off:off + w], ups[:, 0:w])

                # ---- per segment: R tanh passes + e matmuls + exp -------
                slot0 = 0
                for si in range(nseg if STAGE >= 1 else 0):
                    s0, s1 = int(seg_off[si]), int(seg_off[si + 1])
                    Th = s1 - s0
                    e_ps = PE_.tile([P, SEG_CAP], F32, tag="eps")
                    for r in range(R):
                        sc = SC.tile([A, SEG_CAP], BF16, tag="sc")
                        nc.scalar.activation(
                            sc[:, 0:Th], uhT[:, s0:s1], AF.Tanh,
                            bias=bvals[:, r:r + 1], scale=1.0)
                        for off, w in (chunks(0, Th) if STAGE >= 2 else []):
                            nc.tensor.matmul(
                                e_ps[:, off:off + w],
                                co_sb[:, (r + 1) * P:(r + 2) * P],
                                sc[:, off:off + w],
                                start=(r == 0), stop=False)
                    for off, w in (chunks(0, Th) if STAGE >= 2 else []):
                        nc.tensor.matmul(
                            e_ps[:, off:off + w], co_sb[:, 0:P],
                            uhT[:, s0 + off:s0 + off + w],
                            start=False, stop=False)
                        nc.tensor.matmul(
                            e_ps[:, off:off + w], ones_b[:, 0:P],
                            m_row[:, s0 + off:s0 + off + w],
                            start=False, stop=True)
                    if STAGE >= 2:
                        nc.scalar.activation(
                            e_full[:, s0:s1], e_ps[:, 0:Th], AF.Exp,
                            bias=nbnd[:, 0:1], scale=1.0,
                            accum_out=sumh[:, si:si + 1])

                    # ---- apply for this segment's slots (3 per bank) ----
                    seg_slots = list(range(slot0, slot0 + len(segs[si])))
                    slot0 += len(segs[si])
                    groups = [seg_slots[g0:g0 + 3]
                              for g0 in range(0, len(seg_slots), 3)]
                    if STAGE < 3:
                        groups = []

                    def do_group(grp, ops):
                        for j, i in enumerate(grp):
                            le_i = slot_w[i]
                            atp = PT.tile([128, P], BF16, tag="atp")
                            nc.tensor.transpose(
                                atp[0:le_i, :],
                                e_full[:, slot_off[i]:slot_off[i] + le_i],
                                ident_b[:])
                            nc.vector.tensor_copy(
                                aT_all[0:le_i, i * P:(i + 1) * P], atp[0:le_i, :])
                            nc.tensor.matmul(
                                ops[32 * j:32 * (j + 1), :],
                                aT_all[0:le_i, i * P:(i + 1) * P],
                                x_all[0:le_i, i * D:(i + 1) * D],
                                start=True, stop=True)

                    full = [g for g in groups if len(g) == 3]
                    rem = [g for g in groups if len(g) < 3]
                    last_seg = si == nseg - 1
                    bsz = 3 if last_seg else 5
                    for b0 in range(0, len(full), bsz):
                        batch = full[b0:b0 + bsz]
                        nb = len(batch)
                        osb = OSB.tile([128, 5 * D], BF16, tag="osb", bufs=2)
                        for j, grp in enumerate(batch):
                            ops = PO.tile([128, D], F32, tag="ops")
                            do_group(grp, ops)
                            if last_seg:
                                nc.scalar.activation(
                                    osb[0:96, j * D:(j + 1) * D], ops[0:96, :],
                                    AF.Copy, bias=0.0, scale=1.0)
                            else:
                                nc.vector.tensor_copy(
                                    osb[0:96, j * D:(j + 1) * D], ops[0:96, :])
                        s0 = batch[0][0]
                        nc.sync.dma_start(
                            out_dram.ap().rearrange("s p d -> (s p) d")[
                                s0 * P:s0 * P + nb * 96, :].rearrange(
                                    "(g r) d -> r g d", r=96),
                            osb[0:96, 0:nb * D].rearrange("r (g d) -> r g d", d=D))
                    for grp in rem:
                        ops = PO.tile([128, D], F32, tag="ops")
                        do_group(grp, ops)
                        osb2 = OSB.tile([128, D], BF16, tag="osb2", bufs=2)
                        nc.vector.tensor_copy(osb2[0:32 * len(grp), :],
                                              ops[0:32 * len(grp), :])
                        nc.sync.dma_start(
                            out_dram.ap().rearrange("s p d -> (s p) d")[
                                grp[0] * P:(grp[-1] + 1) * P, :],
                            osb2[0:32 * len(grp), :])

            if DEBUG_DUMP:
                nc.sync.dma_start(uh_dbg[:], uhT[:])
                if STAGE >= 2:
                    nc.sync.dma_start(ef_dbg[:], e_full[:])
            if STAGE >= 2:
                nc.sync.dma_start(den_dram[:], sumh[:])
            else:
                nc.gpsimd.memset(sumh[:], 1.0)
                nc.sync.dma_start(den_dram[:], sumh[:])

    nc.compile()
    return nc


_NC_CACHE = {}
LAST_NC = None


def _get_nc(layout, bound):
    key = (layout, round(float(bound), 3))
    if key not in _NC_CACHE:
        _NC_CACHE[key] = build_kernel(layout, bound)
    return _NC_CACHE[key]


# ---------------------------------------------------------------------------
# host entry point
# ---------------------------------------------------------------------------


def _make_layout(le_prof):
    """Split the descending per-rank profile into segments with padded total
    <= SEG_CAP each, balancing totals; widths padded so each segment total is
    a multiple of 64 (pad added to the last slot of the segment)."""
    tot = int(le_prof.sum())
    nseg = max(2, int(np.ceil(_ceil(tot, 64) / SEG_CAP)))
    # balanced greedy split points on the prefix sums
    pref = np.concatenate([[0], np.cumsum(le_prof)])
    bounds = [0]
    for k in range(1, nseg):
        target = tot * k / nseg
        bounds.append(int(np.argmin(np.abs(pref - target))))
    bounds.append(Q)
    segs = []
    for k in range(nseg):
        wdts = [int(v) for v in le_prof[bounds[k]:bounds[k + 1]]]
        pad = int(_ceil(sum(wdts), 64) - sum(wdts))
        wdts[-1] += pad
        assert sum(wdts) <= SEG_CAP
        segs.append(tuple(wdts))
    return tuple(segs)


def kernel(exp_tokens, exp_mask, s_j, req_mask, Ws_w, Ws_b, U_w, v_w):
    """Full-input entry point: shard over B across 8 cores, gather output."""
    from concourse.bass_utils import run_bass_kernel_spmd

    exp_tokens = np.asarray(exp_tokens, dtype=np.float32)
    exp_mask = np.asarray(exp_mask, dtype=np.int32)
    s_j = np.asarray(s_j, dtype=np.float32)
    req_mask = np.asarray(req_mask, dtype=np.int32)
    Ws_w = np.asarray(Ws_w, dtype=np.float32)
    Ws_b = np.asarray(Ws_b, dtype=np.float32)
    U_w = np.asarray(U_w, dtype=np.float32)
    v_w = np.asarray(v_w, dtype=np.float32)

    # ---- per-rank compaction profile ------------------------------------
    counts = exp_mask.sum(axis=2)                      # [B, Q]
    order = np.argsort(-counts, axis=1, kind="stable")
    sorted_counts = np.take_along_axis(counts, order, axis=1)
    le_prof = sorted_counts.max(axis=0)                # [Q]
    layout = _make_layout(le_prof)
    slot_w = [w for s in layout for w in s]
    slot_off = np.concatenate([[0], np.cumsum(slot_w)]).astype(int)
    T = int(slot_off[-1])

    # ---- compacted x + mask row ----------------------------------------
    x_c = np.zeros((B, T, D), dtype=np.float32)
    m_row = np.full((B, 1, T), -1e9, dtype=np.float32)
    for b in range(B):
        for i in range(Q):
            qo = order[b, i]
            idx = np.flatnonzero(exp_mask[b, qo])
            n = len(idx)
            o = slot_off[i]
            x_c[b, o:o + n] = exp_tokens[b, qo, idx]
            m_row[b, 0, o:o + n] = 0.0

    # ---- host coefficients ---------------------------------------------
    ws = np.einsum("bpd,ad->bpa", s_j, Ws_w, optimize=True) + Ws_b  # [B,P,A]
    co = _coef_eval(ws) * v_w[0][None, None, None, :]  # [(R+1), B, P, A]
    # stationary layout [A, (R+1)*P], order: linear first, then basis r
    coefs = np.ascontiguousarray(
        np.transpose(co, (1, 3, 0, 2)).reshape(B, A, (R + 1) * P))
    bound = float(np.abs(co[1:]).sum(axis=(0, 3)).max()
                  + 6.0 * np.abs(co[0]).sum(axis=2).max()) + 1.0
    bound = _ceil(bound, 4.0)

    # uwT: [128, DC*A] with uwT[dd, c*A+a] = U_w[a, c*128+dd]
    uwT = np.ascontiguousarray(
        U_w.reshape(A, DC, 128).transpose(2, 1, 0).reshape(128, DC * A))

    nc = _get_nc(layout, bound)
    global LAST_NC
    LAST_NC = nc

    x_bf = x_c.astype(ml_dtypes.bfloat16)
    x_pad = np.zeros((B, 128, Q, D), dtype=np.float32)
    for i in range(Q):
        o, wdt = int(slot_off[i]), slot_w[i]
        x_pad[:, 0:wdt, i, :] = x_c[:, o:o + wdt, :]
    x_pad_bf = x_pad.reshape(B, 128, Q * D).astype(ml_dtypes.bfloat16)
    uwT_bf = uwT.astype(ml_dtypes.bfloat16)
    coefs_bf = coefs.astype(ml_dtypes.bfloat16)
    in_maps = []
    for b in range(B):
        in_maps.append({
            "x": x_bf[b],
            "x_pad": x_pad_bf[b],
            "m_row_in": m_row[b].astype(ml_dtypes.bfloat16),
            "uwT": uwT_bf,
            "coefs": coefs_bf[b],
        })
    res = run_bass_kernel_spmd(nc, in_maps, core_ids=list(range(N_CORES)))

    out = np.empty((B, Q, P, D), dtype=np.float32)
    for b in range(B):
        o_slot = res.results[b]["out"].astype(np.float32)   # [Q, P, D]
        den = res.results[b]["den"].astype(np.float64).sum(axis=1)  # [P]
        scale = (req_mask[b].astype(np.float64) / (den + 1e-300)).astype(np.float32)
        o_slot *= scale[None, :, None]
        out[b, order[b]] = o_slot
    return out


# revision 21
# speedup vs baseline: 1.2713x; 1.2713x over previous
"""Trainium2 Bass kernel for nn_AbilityGammaAttention.

Reference computation (per batch b):
    ws = s_j @ Ws_w.T + Ws_b                      # (P, A)
    uh = exp_tokens @ U_w.T                       # (Q, LE, A)
    e[q,p,t] = v . tanh(uh[q,t,:] + ws[p,:])      # (Q, P, LE)
    e masked by exp_mask (tokens), joint softmax over (Q, LE) per (b, p)
    out[q,p,:] = sum_t a[q,p,t] * exp_tokens[q,t,:], zeroed where req_mask[p]==0

Sharding: data-parallel over B across the 8 NeuronCores (batch b -> core b).

Key idea (replaces the per-p tanh loop of the previous version): expand the
shifted-tanh family in a fixed basis,

    tanh(u + w) ~= c0(w) + clin(w)*u + sum_r c_r(w) * tanh(u + b_r),

with R=12 fixed shifts b_r. The c*(w) coefficient functions are least-squares
fits (precomputed on a w-grid at import; Gaussian-weighted in u with a uniform
floor for tail control). Since ws = s_j@Ws_w.T + Ws_b is host-computable, the
host evaluates the coefficients at the actual w values and uploads, per core,
stationary matrices S_r[a,p] = v_a * c_r(ws[p,a]). On device:

    e[p,t] = sum_r (S_r^T @ tanh(uhT + b_r))[p,t] + (S_lin^T @ uhT)[p,t] + mask

i.e. R ScalarE tanh passes over [A, T] + (R+2) PE matmuls, instead of P=32
tanh passes. The c0 term is a per-p constant -> cancels in the softmax over t
(the denominators come from the Exp pass's accum_out and the normalization +
req_mask is applied on the host).

Other structure:
  - Host-side token compaction with a per-rank profile: each batch's queries
    are sorted by unmasked-token count; slot i is padded to the max count at
    rank i across batches (shared static shape, ~18% fewer tokens than
    uniform padding). Padding slots are masked via an additive (m-1)*1e9
    rank-1 matmul, exactly like the reference -1e9 masking.
  - All heavy dataflow in bf16: x is uploaded bf16 and XBAR-transpose-DMA'd
    into xT (no on-device transposes for uh), uh/tanh/coefficients/apply all
    bf16 (f32 PSUM accumulation).
  - Output: per-slot apply matmuls pack 3 slots per PSUM bank (base
    partitions 0/32/64), evacuated bf16 and DMA'd out unnormalized; the host
    divides by the denominators and applies req_mask.
"""

import sys

if "/opt/trn_rl_repo" not in sys.path:
    sys.path.insert(0, "/opt/trn_rl_repo")

import numpy as np
import ml_dtypes

import concourse.bacc as bacc
import concourse.mybir as mybir
from concourse.masks import make_identity
from concourse.tile import TileContext

F32 = mybir.dt.float32
F32R = mybir.dt.float32r
BF16 = mybir.dt.bfloat16
AF = mybir.ActivationFunctionType

B, Q, LE, D, P, A = 8, 32, 128, 512, 32, 128
N_CORES = 8
DC = D // 128
R = 12                       # tanh basis size
DEBUG_DUMP = False
STAGE = 3
import os as _os
KSKIP = set(_os.environ.get('KSKIP','').split(','))  # debug: 0=uh only, 1=+tanh, 2=+e/exp, 3=full
SEG_CAP = 1536               # max tokens per segment (3 PSUM banks of f32)

# ---------------------------------------------------------------------------
# basis fit (data-independent; computed once at import)
# ---------------------------------------------------------------------------


def _build_fit(r=R, ridge=1e-6):
    b_r = np.linspace(-4.4, 4.4, r)
    u = np.linspace(-6.5, 6.5, 1301)
    rho = np.exp(-0.5 * u * u) + 0.01
    rho /= rho.sum()
    Phi = np.concatenate(
        [np.ones_like(u)[:, None], u[:, None], np.tanh(u[:, None] + b_r[None, :])],
        axis=1)
    M = Phi.T @ (rho[:, None] * Phi) + ridge * np.eye(r + 2)
    w_grid = np.linspace(-5.0, 5.0, 2001)
    G = np.tanh(u[:, None] + w_grid[None, :])
    C_grid = np.linalg.solve(M, Phi.T @ (rho[:, None] * G))   # [(r+2), Nw]
    return b_r, w_grid, C_grid


_B_R, _W_GRID, _C_GRID = _build_fit()


def _coef_eval(w):
    """Evaluate coefficient functions (rows 1..R+1: linear + R basis) at w."""
    wc = np.clip(w, _W_GRID[0], _W_GRID[-1])
    out = np.empty((R + 1,) + w.shape, dtype=np.float64)
    for i in range(R + 1):
        out[i] = np.interp(wc, _W_GRID, _C_GRID[i + 1])
    return out


def _ceil(x, m):
    return -(-x // m) * m


# ---------------------------------------------------------------------------
# device kernel
# ---------------------------------------------------------------------------


def build_kernel(layout, bound):
    """layout: tuple of segments, each a tuple of slot widths (le_i, padded so
    each segment total is a multiple of 64 -- pad carried by the last slot's
    mask only; slot widths themselves are the DMA/apply sizes)."""
    segs = [list(s) for s in layout]
    seg_T = [sum(s) for s in segs]
    T = sum(seg_T)
    seg_off = np.concatenate([[0], np.cumsum(seg_T)]).astype(int)
    assert all(t % 64 == 0 and t <= SEG_CAP for t in seg_T)
    nseg = len(segs)

    # global slot offsets/widths (in token axis), slot index = (seg, j)
    slot_off = []
    slot_w = []
    for si, s in enumerate(segs):
        o = int(seg_off[si])
        for wdt in s:
            slot_off.append(o)
            slot_w.append(int(wdt))
            o += int(wdt)
    nslots = len(slot_off)
    assert nslots == Q

    nc = bacc.Bacc("TRN2", target_bir_lowering=False, debug=False)

    xt_dram = nc.dram_tensor("xT_in", [128, DC * T], BF16, kind="ExternalInput")
    xp_dram = nc.dram_tensor("x_pad", [128, Q * D], BF16, kind="ExternalInput")
    m_dram = nc.dram_tensor("m_row_in", [1, T], BF16, kind="ExternalInput")
    uwt_dram = nc.dram_tensor("uwT", [128, DC * A], BF16, kind="ExternalInput")
    co_dram = nc.dram_tensor("coefs", [A, (R + 1) * P], BF16, kind="ExternalInput")
    out_dram = nc.dram_tensor("out", [Q, P, D], BF16, kind="ExternalOutput")
    den_dram = nc.dram_tensor("den", [P, nseg], F32, kind="ExternalOutput")
    if DEBUG_DUMP:
        uh_dbg = nc.dram_tensor("uh_dbg", [A, T], BF16, kind="ExternalOutput")
        ef_dbg = nc.dram_tensor("ef_dbg", [P, T], BF16, kind="ExternalOutput")

    def chunks(lo, hi, step=512):
        return [(o, min(step, hi - o)) for o in range(lo, hi, step)]

    with TileContext(nc) as tc:
        with tc.tile_pool(name="live", bufs=1) as L:
            ident = L.tile([128, 128], F32)
            ident_b = L.tile([P, P], BF16)
            x_all = L.tile([128, Q * D], BF16)
            xT = L.tile([128, DC * T], BF16)
            uhT = L.tile([A, T], BF16)
            uwt_sb = L.tile([128, DC * A], BF16)
            co_sb = L.tile([A, (R + 1) * P], BF16)
            m_row = L.tile([1, T], BF16)
            ones_b = L.tile([1, P], BF16)
            e_full = L.tile([P, T], BF16)
            aT_all = L.tile([128, Q * P], BF16)
            sumh = L.tile([P, nseg], F32)
            bvals = L.tile([A, R], F32)
            nbnd = L.tile([P, 1], F32)

            make_identity(nc, ident)
            nc.vector.tensor_copy(ident_b[:], ident[0:P, 0:P])
            nc.gpsimd.memset(ones_b[:], 1.0)
            for r in range(R):
                nc.gpsimd.memset(bvals[:, r:r + 1], float(_B_R[r]))
            nc.gpsimd.memset(nbnd[:], -float(bound))

            # ---- input DMAs (xT first: uh is the critical path) ---------
            # hoist the act-table load: dummy tanh before any DMA lands
            dummy = L.tile([A, 1], BF16)
            nc.scalar.activation(dummy[:], bvals[:, 0:1], AF.Tanh,
                                 bias=0.0, scale=1.0)

            def xt_seg(si):
                s0, s1 = int(seg_off[si]), int(seg_off[si + 1])
                nc.sync.dma_start(
                    xT[:, 0:DC * T].rearrange("r (c t) -> r c t", t=T)[:, :, s0:s1],
                    xt_dram.ap().rearrange("r (c t) -> r c t", t=T)[:, :, s0:s1])

            def xpad_seg(si):
                sl0 = sum(len(s) for s in segs[:si])
                sl1 = sl0 + len(segs[si])
                nc.sync.dma_start(x_all[:, sl0 * D:sl1 * D],
                                  xp_dram.ap()[:, sl0 * D:sl1 * D])

            xt_seg(0)
            nc.sync.dma_start(uwt_sb[:], uwt_dram[:])
            if 'co' not in KSKIP:
                nc.sync.dma_start(co_sb[:], co_dram[:])
            if 'm' not in KSKIP:
                nc.sync.dma_start(m_row[:], m_dram[:])
            for si in range(1, nseg):
                xt_seg(si)
            for si in range(nseg):
                xpad_seg(si)

            with (
                tc.tile_pool(name="scp", bufs=3) as SC,
                tc.tile_pool(name="osp", bufs=3) as OSB,
                tc.tile_pool(name="ups", bufs=1, space="PSUM") as UPS,
                tc.tile_pool(name="pse", bufs=1, space="PSUM") as PE_,
                tc.tile_pool(name="pst", bufs=2, space="PSUM") as PT,
                tc.tile_pool(name="pso", bufs=2, space="PSUM") as PO,
            ):
                # ---- uh for all tokens (segment-aligned chunks) ---------
                uh_chunks = []
                for si in range(nseg):
                    uh_chunks += chunks(int(seg_off[si]), int(seg_off[si + 1]))
                for off, w in uh_chunks:
                    ups = UPS.tile([A, 512], F32, tag="ups")
                    for c in range(DC):
                        nc.tensor.matmul(
                            ups[:, 0:w], uwt_sb[:, c * A:(c + 1) * A],
                            xT[:, c * T + off:c * T + off + w],
                            start=(c == 0), stop=(c == DC - 1))
                    nc.vector.tensor_copy(uhT[:, off:off + w], ups[:, 0:w])

                # ---- per segment: R tanh passes + e matmuls + exp -------
                slot0 = 0
                for si in range(nseg if STAGE >= 1 else 0):
                    s0, s1 = int(seg_off[si]), int(seg_off[si + 1])
                    Th = s1 - s0
                    e_ps = PE_.tile([P, SEG_CAP], F32, tag="eps")
                    for r in range(R):
                        sc = SC.tile([A, SEG_CAP], BF16, tag="sc")
                        nc.scalar.activation(
                            sc[:, 0:Th], uhT[:, s0:s1], AF.Tanh,
                            bias=bvals[:, r:r + 1], scale=1.0)
                        for off, w in (chunks(0, Th) if STAGE >= 2 else []):
                            nc.tensor.matmul(
                                e_ps[:, off:off + w],
                                co_sb[:, (r + 1) * P:(r + 2) * P],
                                sc[:, off:off + w],
                                start=(r == 0), stop=False)
                    for off, w in (chunks(0, Th) if STAGE >= 2 else []):
                        nc.tensor.matmul(
                            e_ps[:, off:off + w], co_sb[:, 0:P],
                            uhT[:, s0 + off:s0 + off + w],
                            start=False, stop=False)
                        nc.tensor.matmul(
                            e_ps[:, off:off + w], ones_b[:, 0:P],
                            m_row[:, s0 + off:s0 + off + w],
                            start=False, stop=True)
                    if STAGE >= 2:
                        nc.scalar.activation(
                            e_full[:, s0:s1], e_ps[:, 0:Th], AF.Exp,
                            bias=nbnd[:, 0:1], scale=1.0,
                            accum_out=sumh[:, si:si + 1])

                    # ---- apply for this segment's slots (3 per bank) ----
                    seg_slots = list(range(slot0, slot0 + len(segs[si])))
                    slot0 += len(segs[si])
                    groups = [seg_slots[g0:g0 + 3]
                              for g0 in range(0, len(seg_slots), 3)]
                    if STAGE < 3:
                        groups = []

                    def do_group(grp, ops):
                        for j, i in enumerate(grp):
                            le_i = slot_w[i]
                            atp = PT.tile([128, P], BF16, tag="atp")
                            nc.tensor.transpose(
                                atp[0:le_i, :],
                                e_full[:, slot_off[i]:slot_off[i] + le_i],
                                ident_b[:])
                            nc.vector.tensor_copy(
                                aT_all[0:le_i, i * P:(i + 1) * P], atp[0:le_i, :])
                            nc.tensor.matmul(
                                ops[32 * j:32 * (j + 1), :],
                                aT_all[0:le_i, i * P:(i + 1) * P],
                                x_all[0:le_i, i * D:(i + 1) * D],
                                start=True, stop=True)

                    full = [g for g in groups if len(g) == 3]
                    rem = [g for g in groups if len(g) < 3]
                    last_seg = si == nseg - 1
                    bsz = 1 if last_seg else 5
                    for b0 in range(0, len(full), bsz):
                        batch = full[b0:b0 + bsz]
                        nb = len(batch)
                        osb = OSB.tile([128, 5 * D], BF16, tag="osb", bufs=2)
                        for j, grp in enumerate(batch):
                            ops = PO.tile([128, D], F32, tag="ops")
                            do_group(grp, ops)
                            if last_seg:
                                nc.scalar.activation(
                                    osb[0:96, j * D:(j + 1) * D], ops[0:96, :],
                                    AF.Copy, bias=0.0, scale=1.0)
                            else:
                                nc.vector.tensor_copy(
                                    osb[0:96, j * D:(j + 1) * D], ops[0:96, :])
                        s0 = batch[0][0]
                        nc.sync.dma_start(
                            out_dram.ap().rearrange("s p d -> (s p) d")[
                                s0 * P:s0 * P + nb * 96, :].rearrange(
                                    "(g r) d -> r g d", r=96),
                            osb[0:96, 0:nb * D].rearrange("r (g d) -> r g d", d=D))
                    for grp in rem:
                        ops = PO.tile([128, D], F32, tag="ops")
                        do_group(grp, ops)
                        osb2 = OSB.tile([128, D], BF16, tag="osb2", bufs=2)
                        nc.vector.tensor_copy(osb2[0:32 * len(grp), :],
                                              ops[0:32 * len(grp), :])
                        nc.sync.dma_start(
                            out_dram.ap().rearrange("s p d -> (s p) d")[
                                grp[0] * P:(grp[-1] + 1) * P, :],
                            osb2[0:32 * len(grp), :])

            if DEBUG_DUMP:
                nc.sync.dma_start(uh_dbg[:], uhT[:])
                if STAGE >= 2:
                    nc.sync.dma_start(ef_dbg[:], e_full[:])
            if STAGE >= 2:
                nc.sync.dma_start(den_dram[:], sumh[:])
            else:
                nc.gpsimd.memset(sumh[:], 1.0)
                nc.sync.dma_start(den_dram[:], sumh[:])

    nc.compile()
    return nc


_NC_CACHE = {}
LAST_NC = None


def _get_nc(layout, bound):
    key = (layout, round(float(bound), 3))
    if key not in _NC_CACHE:
        _NC_CACHE[key] = build_kernel(layout, bound)
    return _NC_CACHE[key]


# ---------------------------------------------------------------------------
# host entry point
# ---------------------------------------------------------------------------


SEG_FRACS = (0.2, 0.6)  # cumulative split fractions (None = balanced)


def _make_layout(le_prof):
    """Split the descending per-rank profile into segments with padded total
    <= SEG_CAP each, balancing totals; widths padded so each segment total is
    a multiple of 64 (pad added to the last slot of the segment)."""
    tot = int(le_prof.sum())
    pref = np.concatenate([[0], np.cumsum(le_prof)])
    if SEG_FRACS is not None:
        bounds = [0] + [int(np.argmin(np.abs(pref - tot * f)))
                        for f in SEG_FRACS] + [Q]
        nseg = len(SEG_FRACS) + 1
    else:
        nseg = max(2, int(np.ceil(_ceil(tot, 64) / SEG_CAP)))
        bounds = [0]
        for k in range(1, nseg):
            target = tot * k / nseg
            bounds.append(int(np.argmin(np.abs(pref - target))))
        bounds.append(Q)
    def build(bnds):
        segs = []
        for k in range(len(bnds) - 1):
            wdts = [int(v) for v in le_prof[bnds[k]:bnds[k + 1]]]
            if not wdts:
                return None
            wdts[-1] += int(_ceil(sum(wdts), 64) - sum(wdts))
            if sum(wdts) > SEG_CAP:
                return None
            segs.append(tuple(wdts))
        return tuple(segs)

    segs = build(bounds)
    if segs is None:
        # fallback: balanced split into as many segments as needed
        nseg = max(2, int(np.ceil(_ceil(tot, 64) / (SEG_CAP - 128))))
        bounds = [0] + [int(np.argmin(np.abs(pref - tot * k / nseg)))
                        for k in range(1, nseg)] + [Q]
        segs = build(bounds)
    assert segs is not None
    return segs


def kernel(exp_tokens, exp_mask, s_j, req_mask, Ws_w, Ws_b, U_w, v_w):
    """Full-input entry point: shard over B across 8 cores, gather output."""
    from concourse.bass_utils import run_bass_kernel_spmd

    exp_tokens = np.asarray(exp_tokens, dtype=np.float32)
    exp_mask = np.asarray(exp_mask, dtype=np.int32)
    s_j = np.asarray(s_j, dtype=np.float32)
    req_mask = np.asarray(req_mask, dtype=np.int32)
    Ws_w = np.asarray(Ws_w, dtype=np.float32)
    Ws_b = np.asarray(Ws_b, dtype=np.float32)
    U_w = np.asarray(U_w, dtype=np.float32)
    v_w = np.asarray(v_w, dtype=np.float32)

    # ---- per-rank compaction profile ------------------------------------
    counts = exp_mask.sum(axis=2)                      # [B, Q]
    order = np.argsort(-counts, axis=1, kind="stable")
    sorted_counts = np.take_along_axis(counts, order, axis=1)
    le_prof = sorted_counts.max(axis=0)                # [Q]
    layout = _make_layout(le_prof)
    slot_w = [w for s in layout for w in s]
    slot_off = np.concatenate([[0], np.cumsum(slot_w)]).astype(int)
    T = int(slot_off[-1])

    # ---- compacted x + mask row ----------------------------------------
    x_c = np.zeros((B, T, D), dtype=np.float32)
    m_row = np.full((B, 1, T), -1e9, dtype=np.float32)
    for b in range(B):
        for i in range(Q):
            qo = order[b, i]
            idx = np.flatnonzero(exp_mask[b, qo])
            n = len(idx)
            o = slot_off[i]
            x_c[b, o:o + n] = exp_tokens[b, qo, idx]
            m_row[b, 0, o:o + n] = 0.0

    # ---- host coefficients ---------------------------------------------
    ws = np.einsum("bpd,ad->bpa", s_j, Ws_w, optimize=True) + Ws_b  # [B,P,A]
    co = _coef_eval(ws) * v_w[0][None, None, None, :]  # [(R+1), B, P, A]
    # stationary layout [A, (R+1)*P], order: linear first, then basis r
    coefs = np.ascontiguousarray(
        np.transpose(co, (1, 3, 0, 2)).reshape(B, A, (R + 1) * P))
    bound = float(np.abs(co[1:]).sum(axis=(0, 3)).max()
                  + 6.0 * np.abs(co[0]).sum(axis=2).max()) + 1.0
    bound = _ceil(bound, 4.0)

    # uwT: [128, DC*A] with uwT[dd, c*A+a] = U_w[a, c*128+dd]
    uwT = np.ascontiguousarray(
        U_w.reshape(A, DC, 128).transpose(2, 1, 0).reshape(128, DC * A))

    nc = _get_nc(layout, bound)
    global LAST_NC
    LAST_NC = nc

    xT_host = np.ascontiguousarray(
        x_c.reshape(B, T, DC, 128).transpose(0, 3, 2, 1)).reshape(B, 128, DC * T)
    xT_bf = xT_host.astype(ml_dtypes.bfloat16)
    x_pad = np.zeros((B, 128, Q, D), dtype=np.float32)
    for i in range(Q):
        o, wdt = int(slot_off[i]), slot_w[i]
        x_pad[:, 0:wdt, i, :] = x_c[:, o:o + wdt, :]
    x_pad_bf = x_pad.reshape(B, 128, Q * D).astype(ml_dtypes.bfloat16)
    uwT_bf = uwT.astype(ml_dtypes.bfloat16)
    coefs_bf = coefs.astype(ml_dtypes.bfloat16)
    in_maps = []
    for b in range(B):
        in_maps.append({
            "xT_in": xT_bf[b],
            "x_pad": x_pad_bf[b],
            "m_row_in": m_row[b].astype(ml_dtypes.bfloat16),
            "uwT": uwT_bf,
            "coefs": coefs_bf[b],
        })
    res = run_bass_kernel_spmd(nc, in_maps, core_ids=list(range(N_CORES)))

    out = np.empty((B, Q, P, D), dtype=np.float32)
    for b in range(B):
        o_slot = res.results[b]["out"].astype(np.float32)   # [Q, P, D]
        den = res.results[b]["den"].astype(np.float64).sum(axis=1)  # [P]
        scale = (req_mask[b].astype(np.float64) / (den + 1e-300)).astype(np.float32)
        o_slot *= scale[None, :, None]
        out[b, order[b]] = o_slot
    return out


# revision 22
# speedup vs baseline: 1.2864x; 1.0119x over previous
"""Trainium2 Bass kernel for nn_AbilityGammaAttention.

Reference computation (per batch b):
    ws = s_j @ Ws_w.T + Ws_b                      # (P, A)
    uh = exp_tokens @ U_w.T                       # (Q, LE, A)
    e[q,p,t] = v . tanh(uh[q,t,:] + ws[p,:])      # (Q, P, LE)
    e masked by exp_mask (tokens), joint softmax over (Q, LE) per (b, p)
    out[q,p,:] = sum_t a[q,p,t] * exp_tokens[q,t,:], zeroed where req_mask[p]==0

Sharding: data-parallel over B across the 8 NeuronCores (batch b -> core b).

Key idea (replaces the per-p tanh loop of the previous version): expand the
shifted-tanh family in a fixed basis,

    tanh(u + w) ~= c0(w) + clin(w)*u + sum_r c_r(w) * tanh(u + b_r),

with R=12 fixed shifts b_r. The c*(w) coefficient functions are least-squares
fits (precomputed on a w-grid at import; Gaussian-weighted in u with a uniform
floor for tail control). Since ws = s_j@Ws_w.T + Ws_b is host-computable, the
host evaluates the coefficients at the actual w values and uploads, per core,
stationary matrices S_r[a,p] = v_a * c_r(ws[p,a]). On device:

    e[p,t] = sum_r (S_r^T @ tanh(uhT + b_r))[p,t] + (S_lin^T @ uhT)[p,t] + mask

i.e. R ScalarE tanh passes over [A, T] + (R+2) PE matmuls, instead of P=32
tanh passes. The c0 term is a per-p constant -> cancels in the softmax over t
(the denominators come from the Exp pass's accum_out and the normalization +
req_mask is applied on the host).

Other structure:
  - Host-side token compaction with a per-rank profile: each batch's queries
    are sorted by unmasked-token count; slot i is padded to the max count at
    rank i across batches (shared static shape, ~18% fewer tokens than
    uniform padding). Padding slots are masked via an additive (m-1)*1e9
    rank-1 matmul, exactly like the reference -1e9 masking.
  - All heavy dataflow in bf16: x is uploaded bf16 and XBAR-transpose-DMA'd
    into xT (no on-device transposes for uh), uh/tanh/coefficients/apply all
    bf16 (f32 PSUM accumulation).
  - Output: per-slot apply matmuls pack 3 slots per PSUM bank (base
    partitions 0/32/64), evacuated bf16 and DMA'd out unnormalized; the host
    divides by the denominators and applies req_mask.
"""

import sys

if "/opt/trn_rl_repo" not in sys.path:
    sys.path.insert(0, "/opt/trn_rl_repo")

import numpy as np
import ml_dtypes

import concourse.bacc as bacc
import concourse.mybir as mybir
from concourse.masks import make_identity
from concourse.tile import TileContext

F32 = mybir.dt.float32
F32R = mybir.dt.float32r
BF16 = mybir.dt.bfloat16
AF = mybir.ActivationFunctionType

B, Q, LE, D, P, A = 8, 32, 128, 512, 32, 128
N_CORES = 8
DC = D // 128
R = 12                       # tanh basis size
DEBUG_DUMP = False
STAGE = 3
import os as _os
KSKIP = set(_os.environ.get('KSKIP','').split(','))  # debug: 0=uh only, 1=+tanh, 2=+e/exp, 3=full
SEG_CAP = 1536               # max tokens per segment (3 PSUM banks of f32)

# ---------------------------------------------------------------------------
# basis fit (data-independent; computed once at import)
# ---------------------------------------------------------------------------


def _build_fit(r=R, ridge=1e-6):
    b_r = np.linspace(-4.4, 4.4, r)
    u = np.linspace(-6.5, 6.5, 1301)
    rho = np.exp(-0.5 * u * u) + 0.01
    rho /= rho.sum()
    Phi = np.concatenate(
        [np.ones_like(u)[:, None], u[:, None], np.tanh(u[:, None] + b_r[None, :])],
        axis=1)
    M = Phi.T @ (rho[:, None] * Phi) + ridge * np.eye(r + 2)
    w_grid = np.linspace(-5.0, 5.0, 2001)
    G = np.tanh(u[:, None] + w_grid[None, :])
    C_grid = np.linalg.solve(M, Phi.T @ (rho[:, None] * G))   # [(r+2), Nw]
    return b_r, w_grid, C_grid


_B_R, _W_GRID, _C_GRID = _build_fit()


def _coef_eval(w):
    """Evaluate coefficient functions (rows 1..R+1: linear + R basis) at w."""
    wc = np.clip(w, _W_GRID[0], _W_GRID[-1])
    out = np.empty((R + 1,) + w.shape, dtype=np.float64)
    for i in range(R + 1):
        out[i] = np.interp(wc, _W_GRID, _C_GRID[i + 1])
    return out


def _ceil(x, m):
    return -(-x // m) * m


# ---------------------------------------------------------------------------
# device kernel
# ---------------------------------------------------------------------------


def build_kernel(layout, bound):
    """layout: tuple of segments, each a tuple of slot widths (le_i, padded so
    each segment total is a multiple of 64 -- pad carried by the last slot's
    mask only; slot widths themselves are the DMA/apply sizes)."""
    def chunks_of(n, step=512):
        return [(o, min(step, n - o)) for o in range(0, n, step)]

    segs = [list(s) for s in layout]
    seg_T = [sum(s) for s in segs]
    T = sum(seg_T)
    seg_off = np.concatenate([[0], np.cumsum(seg_T)]).astype(int)
    assert all(t % 64 == 0 and t <= SEG_CAP for t in seg_T)
    nseg = len(segs)

    # global slot offsets/widths (in token axis), slot index = (seg, j)
    slot_off = []
    slot_w = []
    for si, s in enumerate(segs):
        o = int(seg_off[si])
        for wdt in s:
            slot_off.append(o)
            slot_w.append(int(wdt))
            o += int(wdt)
    nslots = len(slot_off)
    assert nslots == Q

    nchunks = sum(len(chunks_of(int(seg_off[i + 1]) - int(seg_off[i])))
                  for i in range(nseg))
    CHN = [0]

    nc = bacc.Bacc("TRN2", target_bir_lowering=False, debug=False)

    xt_dram = nc.dram_tensor("xT_in", [128, DC * T], BF16, kind="ExternalInput")
    xp_dram = nc.dram_tensor("x_pad", [128, Q * D], BF16, kind="ExternalInput")
    m_dram = nc.dram_tensor("m_row_in", [1, T], BF16, kind="ExternalInput")
    uwt_dram = nc.dram_tensor("uwT", [128, DC * A], BF16, kind="ExternalInput")
    co_dram = nc.dram_tensor("coefs", [A, (R + 1) * P], BF16, kind="ExternalInput")
    out_dram = nc.dram_tensor("out", [Q, P, D], BF16, kind="ExternalOutput")
    den_dram = nc.dram_tensor("den", [P, nchunks], F32, kind="ExternalOutput")
    if DEBUG_DUMP:
        uh_dbg = nc.dram_tensor("uh_dbg", [A, T], BF16, kind="ExternalOutput")
        ef_dbg = nc.dram_tensor("ef_dbg", [P, T], BF16, kind="ExternalOutput")

    def chunks(lo, hi, step=512):
        return [(o, min(step, hi - o)) for o in range(lo, hi, step)]

    with TileContext(nc) as tc:
        with tc.tile_pool(name="live", bufs=1) as L:
            ident = L.tile([128, 128], F32)
            ident_b = L.tile([P, P], BF16)
            x_all = L.tile([128, Q * D], BF16)
            xT = L.tile([128, DC * T], BF16)
            uhT = L.tile([A, T], BF16)
            uwt_sb = L.tile([128, DC * A], BF16)
            co_sb = L.tile([A, (R + 1) * P], BF16)
            m_row = L.tile([1, T], BF16)
            ones_b = L.tile([1, P], BF16)
            e_full = L.tile([P, T], BF16)
            aT_all = L.tile([128, Q * P], BF16)
            sumh = L.tile([P, nchunks], F32)
            bvals = L.tile([A, R], F32)
            nbnd = L.tile([P, 1], F32)

            make_identity(nc, ident)
            nc.vector.tensor_copy(ident_b[:], ident[0:P, 0:P])
            nc.gpsimd.memset(ones_b[:], 1.0)
            for r in range(R):
                nc.gpsimd.memset(bvals[:, r:r + 1], float(_B_R[r]))
            nc.gpsimd.memset(nbnd[:], -float(bound))

            # ---- input DMAs (xT first: uh is the critical path) ---------
            # hoist the act-table load: dummy tanh before any DMA lands
            dummy = L.tile([A, 1], BF16)
            nc.scalar.activation(dummy[:], bvals[:, 0:1], AF.Tanh,
                                 bias=0.0, scale=1.0)

            def xt_seg(si):
                s0, s1 = int(seg_off[si]), int(seg_off[si + 1])
                nc.sync.dma_start(
                    xT[:, 0:DC * T].rearrange("r (c t) -> r c t", t=T)[:, :, s0:s1],
                    xt_dram.ap().rearrange("r (c t) -> r c t", t=T)[:, :, s0:s1])

            def xpad_seg(si):
                sl0 = sum(len(s) for s in segs[:si])
                sl1 = sl0 + len(segs[si])
                nc.sync.dma_start(x_all[:, sl0 * D:sl1 * D],
                                  xp_dram.ap()[:, sl0 * D:sl1 * D])

            xt_seg(0)
            nc.sync.dma_start(uwt_sb[:], uwt_dram[:])
            if 'co' not in KSKIP:
                nc.sync.dma_start(co_sb[:], co_dram[:])
            if 'm' not in KSKIP:
                nc.sync.dma_start(m_row[:], m_dram[:])
            for si in range(1, nseg):
                xt_seg(si)
            for si in range(nseg):
                xpad_seg(si)

            with (
                tc.tile_pool(name="scp", bufs=6) as SC,
                tc.tile_pool(name="osp", bufs=3) as OSB,
                tc.tile_pool(name="ups", bufs=1, space="PSUM") as UPS,
                tc.tile_pool(name="pse", bufs=1, space="PSUM") as PE_,
                tc.tile_pool(name="pst", bufs=2, space="PSUM") as PT,
                tc.tile_pool(name="pso", bufs=2, space="PSUM") as PO,
            ):
                # ---- uh for all tokens (segment-aligned chunks) ---------
                uh_chunks = []
                for si in range(nseg):
                    uh_chunks += chunks(int(seg_off[si]), int(seg_off[si + 1]))
                for off, w in uh_chunks:
                    ups = UPS.tile([A, 512], F32, tag="ups")
                    for c in range(DC):
                        nc.tensor.matmul(
                            ups[:, 0:w], uwt_sb[:, c * A:(c + 1) * A],
                            xT[:, c * T + off:c * T + off + w],
                            start=(c == 0), stop=(c == DC - 1))
                    nc.vector.tensor_copy(uhT[:, off:off + w], ups[:, 0:w])

                # ---- per segment: R tanh passes + e matmuls + exp -------
                slot0 = 0
                for si in range(nseg if STAGE >= 1 else 0):
                    s0, s1 = int(seg_off[si]), int(seg_off[si + 1])
                    Th = s1 - s0
                    e_ps = PE_.tile([P, SEG_CAP], F32, tag="eps")
                    for r in range(R):
                        sc = SC.tile([A, SEG_CAP], BF16, tag="sc")
                        nc.scalar.activation(
                            sc[:, 0:Th], uhT[:, s0:s1], AF.Tanh,
                            bias=bvals[:, r:r + 1], scale=1.0)
                        for off, w in (chunks(0, Th) if STAGE >= 2 else []):
                            nc.tensor.matmul(
                                e_ps[:, off:off + w],
                                co_sb[:, (r + 1) * P:(r + 2) * P],
                                sc[:, off:off + w],
                                start=(r == 0), stop=False)
                    seg_slots = list(range(slot0, slot0 + len(segs[si])))
                    slot0 += len(segs[si])
                    for off, w in (chunks(0, Th) if STAGE >= 2 else []):
                        nc.tensor.matmul(
                            e_ps[:, off:off + w], co_sb[:, 0:P],
                            uhT[:, s0 + off:s0 + off + w],
                            start=False, stop=False)
                        nc.tensor.matmul(
                            e_ps[:, off:off + w], ones_b[:, 0:P],
                            m_row[:, s0 + off:s0 + off + w],
                            start=False, stop=True)
                        nc.scalar.activation(
                            e_full[:, s0 + off:s0 + off + w],
                            e_ps[:, off:off + w], AF.Exp,
                            bias=nbnd[:, 0:1], scale=1.0,
                            accum_out=sumh[:, CHN[0]:CHN[0] + 1])
                        CHN[0] += 1
                    groups = [seg_slots[g0:g0 + 3]
                              for g0 in range(0, len(seg_slots), 3)]
                    if STAGE < 3:
                        groups = []

                    def do_group(grp, ops):
                        for j, i in enumerate(grp):
                            le_i = slot_w[i]
                            atp = PT.tile([128, P], BF16, tag="atp")
                            nc.tensor.transpose(
                                atp[0:le_i, :],
                                e_full[:, slot_off[i]:slot_off[i] + le_i],
                                ident_b[:])
                            nc.vector.tensor_copy(
                                aT_all[0:le_i, i * P:(i + 1) * P], atp[0:le_i, :])
                            nc.tensor.matmul(
                                ops[32 * j:32 * (j + 1), :],
                                aT_all[0:le_i, i * P:(i + 1) * P],
                                x_all[0:le_i, i * D:(i + 1) * D],
                                start=True, stop=True)

                    full = [g for g in groups if len(g) == 3]
                    rem = [g for g in groups if len(g) < 3]
                    last_seg = si == nseg - 1
                    bsz = 1 if last_seg else 5
                    for b0 in range(0, len(full), bsz):
                        batch = full[b0:b0 + bsz]
                        nb = len(batch)
                        osb = OSB.tile([128, 5 * D], BF16, tag="osb", bufs=4)
                        for j, grp in enumerate(batch):
                            ops = PO.tile([128, D], F32, tag="ops")
                            do_group(grp, ops)
                            if last_seg:
                                nc.scalar.activation(
                                    osb[0:96, j * D:(j + 1) * D], ops[0:96, :],
                                    AF.Copy, bias=0.0, scale=1.0)
                            else:
                                nc.vector.tensor_copy(
                                    osb[0:96, j * D:(j + 1) * D], ops[0:96, :])
                        s0 = batch[0][0]
                        nc.sync.dma_start(
                            out_dram.ap().rearrange("s p d -> (s p) d")[
                                s0 * P:s0 * P + nb * 96, :].rearrange(
                                    "(g r) d -> r g d", r=96),
                            osb[0:96, 0:nb * D].rearrange("r (g d) -> r g d", d=D))
                    for grp in rem:
                        ops = PO.tile([128, D], F32, tag="ops")
                        do_group(grp, ops)
                        osb2 = OSB.tile([128, D], BF16, tag="osb2", bufs=2)
                        nc.vector.tensor_copy(osb2[0:32 * len(grp), :],
                                              ops[0:32 * len(grp), :])
                        nc.sync.dma_start(
                            out_dram.ap().rearrange("s p d -> (s p) d")[
                                grp[0] * P:(grp[-1] + 1) * P, :],
                            osb2[0:32 * len(grp), :])

            if DEBUG_DUMP:
                nc.sync.dma_start(uh_dbg[:], uhT[:])
                if STAGE >= 2:
                    nc.sync.dma_start(ef_dbg[:], e_full[:])
            if STAGE >= 2:
                nc.sync.dma_start(den_dram[:], sumh[:])
            else:
                nc.gpsimd.memset(sumh[:], 1.0)
                nc.sync.dma_start(den_dram[:], sumh[:])

    nc.compile()
    return nc


_NC_CACHE = {}
LAST_NC = None


def _get_nc(layout, bound):
    key = (layout, round(float(bound), 3))
    if key not in _NC_CACHE:
        _NC_CACHE[key] = build_kernel(layout, bound)
    return _NC_CACHE[key]


# ---------------------------------------------------------------------------
# host entry point
# ---------------------------------------------------------------------------


SEG_FRACS = (0.2, 0.6)  # cumulative split fractions (None = balanced)


def _make_layout(le_prof):
    """Split the descending per-rank profile into segments with padded total
    <= SEG_CAP each, balancing totals; widths padded so each segment total is
    a multiple of 64 (pad added to the last slot of the segment)."""
    tot = int(le_prof.sum())
    pref = np.concatenate([[0], np.cumsum(le_prof)])
    if SEG_FRACS is not None:
        bounds = [0] + [int(np.argmin(np.abs(pref - tot * f)))
                        for f in SEG_FRACS] + [Q]
        nseg = len(SEG_FRACS) + 1
    else:
        nseg = max(2, int(np.ceil(_ceil(tot, 64) / SEG_CAP)))
        bounds = [0]
        for k in range(1, nseg):
            target = tot * k / nseg
            bounds.append(int(np.argmin(np.abs(pref - target))))
        bounds.append(Q)
    def build(bnds):
        segs = []
        for k in range(len(bnds) - 1):
            wdts = [int(v) for v in le_prof[bnds[k]:bnds[k + 1]]]
            if not wdts:
                return None
            wdts[-1] += int(_ceil(sum(wdts), 64) - sum(wdts))
            if sum(wdts) > SEG_CAP:
                return None
            segs.append(tuple(wdts))
        return tuple(segs)

    segs = build(bounds)
    if segs is None:
        # fallback: balanced split into as many segments as needed
        nseg = max(2, int(np.ceil(_ceil(tot, 64) / (SEG_CAP - 128))))
        bounds = [0] + [int(np.argmin(np.abs(pref - tot * k / nseg)))
                        for k in range(1, nseg)] + [Q]
        segs = build(bounds)
    assert segs is not None
    return segs


def kernel(exp_tokens, exp_mask, s_j, req_mask, Ws_w, Ws_b, U_w, v_w):
    """Full-input entry point: shard over B across 8 cores, gather output."""
    from concourse.bass_utils import run_bass_kernel_spmd

    exp_tokens = np.asarray(exp_tokens, dtype=np.float32)
    exp_mask = np.asarray(exp_mask, dtype=np.int32)
    s_j = np.asarray(s_j, dtype=np.float32)
    req_mask = np.asarray(req_mask, dtype=np.int32)
    Ws_w = np.asarray(Ws_w, dtype=np.float32)
    Ws_b = np.asarray(Ws_b, dtype=np.float32)
    U_w = np.asarray(U_w, dtype=np.float32)
    v_w = np.asarray(v_w, dtype=np.float32)

    # ---- per-rank compaction profile ------------------------------------
    counts = exp_mask.sum(axis=2)                      # [B, Q]
    order = np.argsort(-counts, axis=1, kind="stable")
    sorted_counts = np.take_along_axis(counts, order, axis=1)
    le_prof = sorted_counts.max(axis=0)                # [Q]
    layout = _make_layout(le_prof)
    slot_w = [w for s in layout for w in s]
    slot_off = np.concatenate([[0], np.cumsum(slot_w)]).astype(int)
    T = int(slot_off[-1])

    # ---- compacted x + mask row ----------------------------------------
    x_c = np.zeros((B, T, D), dtype=np.float32)
    m_row = np.full((B, 1, T), -1e9, dtype=np.float32)
    for b in range(B):
        for i in range(Q):
            qo = order[b, i]
            idx = np.flatnonzero(exp_mask[b, qo])
            n = len(idx)
            o = slot_off[i]
            x_c[b, o:o + n] = exp_tokens[b, qo, idx]
            m_row[b, 0, o:o + n] = 0.0

    # ---- host coefficients ---------------------------------------------
    ws = np.einsum("bpd,ad->bpa", s_j, Ws_w, optimize=True) + Ws_b  # [B,P,A]
    co = _coef_eval(ws) * v_w[0][None, None, None, :]  # [(R+1), B, P, A]
    # stationary layout [A, (R+1)*P], order: linear first, then basis r
    coefs = np.ascontiguousarray(
        np.transpose(co, (1, 3, 0, 2)).reshape(B, A, (R + 1) * P))
    bound = float(np.abs(co[1:]).sum(axis=(0, 3)).max()
                  + 6.0 * np.abs(co[0]).sum(axis=2).max()) + 1.0
    bound = _ceil(bound, 4.0)

    # uwT: [128, DC*A] with uwT[dd, c*A+a] = U_w[a, c*128+dd]
    uwT = np.ascontiguousarray(
        U_w.reshape(A, DC, 128).transpose(2, 1, 0).reshape(128, DC * A))

    nc = _get_nc(layout, bound)
    global LAST_NC
    LAST_NC = nc

    xT_host = np.ascontiguousarray(
        x_c.reshape(B, T, DC, 128).transpose(0, 3, 2, 1)).reshape(B, 128, DC * T)
    xT_bf = xT_host.astype(ml_dtypes.bfloat16)
    x_pad = np.zeros((B, 128, Q, D), dtype=np.float32)
    for i in range(Q):
        o, wdt = int(slot_off[i]), slot_w[i]
        x_pad[:, 0:wdt, i, :] = x_c[:, o:o + wdt, :]
    x_pad_bf = x_pad.reshape(B, 128, Q * D).astype(ml_dtypes.bfloat16)
    uwT_bf = uwT.astype(ml_dtypes.bfloat16)
    coefs_bf = coefs.astype(ml_dtypes.bfloat16)
    in_maps = []
    for b in range(B):
        in_maps.append({
            "xT_in": xT_bf[b],
            "x_pad": x_pad_bf[b],
            "m_row_in": m_row[b].astype(ml_dtypes.bfloat16),
            "uwT": uwT_bf,
            "coefs": coefs_bf[b],
        })
    res = run_bass_kernel_spmd(nc, in_maps, core_ids=list(range(N_CORES)))

    out = np.empty((B, Q, P, D), dtype=np.float32)
    for b in range(B):
        o_slot = res.results[b]["out"].astype(np.float32)   # [Q, P, D]
        den = res.results[b]["den"].astype(np.float64).sum(axis=1)  # [P]
        scale = (req_mask[b].astype(np.float64) / (den + 1e-300)).astype(np.float32)
        o_slot *= scale[None, :, None]
        out[b, order[b]] = o_slot
    return out


# revision 24
# speedup vs baseline: 1.4672x; 1.1405x over previous
"""Trainium2 Bass kernel for nn_AbilityGammaAttention.

Reference computation (per batch b):
    ws = s_j @ Ws_w.T + Ws_b                      # (P, A)
    uh = exp_tokens @ U_w.T                       # (Q, LE, A)
    e[q,p,t] = v . tanh(uh[q,t,:] + ws[p,:])      # (Q, P, LE)
    e masked by exp_mask (tokens), joint softmax over (Q, LE) per (b, p)
    out[q,p,:] = sum_t a[q,p,t] * exp_tokens[q,t,:], zeroed where req_mask[p]==0

Sharding: data-parallel over B across the 8 NeuronCores (batch b -> core b).

Key idea (replaces the per-p tanh loop of the previous version): expand the
shifted-tanh family in a fixed basis,

    tanh(u + w) ~= c0(w) + clin(w)*u + sum_r c_r(w) * tanh(u + b_r),

with R=12 fixed shifts b_r. The c*(w) coefficient functions are least-squares
fits (precomputed on a w-grid at import; Gaussian-weighted in u with a uniform
floor for tail control). Since ws = s_j@Ws_w.T + Ws_b is host-computable, the
host evaluates the coefficients at the actual w values and uploads, per core,
stationary matrices S_r[a,p] = v_a * c_r(ws[p,a]). On device:

    e[p,t] = sum_r (S_r^T @ tanh(uhT + b_r))[p,t] + (S_lin^T @ uhT)[p,t] + mask

i.e. R ScalarE tanh passes over [A, T] + (R+2) PE matmuls, instead of P=32
tanh passes. The c0 term is a per-p constant -> cancels in the softmax over t
(the denominators come from the Exp pass's accum_out and the normalization +
req_mask is applied on the host).

Other structure:
  - Host-side token compaction with a per-rank profile: each batch's queries
    are sorted by unmasked-token count; slot i is padded to the max count at
    rank i across batches (shared static shape, ~18% fewer tokens than
    uniform padding). Padding slots are masked via an additive (m-1)*1e9
    rank-1 matmul, exactly like the reference -1e9 masking.
  - All heavy dataflow in bf16: x is uploaded bf16 and XBAR-transpose-DMA'd
    into xT (no on-device transposes for uh), uh/tanh/coefficients/apply all
    bf16 (f32 PSUM accumulation).
  - Output: per-slot apply matmuls pack 3 slots per PSUM bank (base
    partitions 0/32/64), evacuated bf16 and DMA'd out unnormalized; the host
    divides by the denominators and applies req_mask.
"""

import sys

if "/opt/trn_rl_repo" not in sys.path:
    sys.path.insert(0, "/opt/trn_rl_repo")

import numpy as np
import ml_dtypes

import concourse.bacc as bacc
import concourse.mybir as mybir
from concourse.masks import make_identity
from concourse.tile import TileContext

F32 = mybir.dt.float32
F32R = mybir.dt.float32r
BF16 = mybir.dt.bfloat16
AF = mybir.ActivationFunctionType

B, Q, LE, D, P, A = 8, 32, 128, 512, 32, 128
N_CORES = 8
DC = D // 128
R = 9                        # tanh basis size
DEBUG_DUMP = False
STAGE = 3
import os as _os
KSKIP = set(_os.environ.get('KSKIP','').split(','))  # debug: 0=uh only, 1=+tanh, 2=+e/exp, 3=full
SEG_CAP = 1536               # max tokens per segment (3 PSUM banks of f32)

# ---------------------------------------------------------------------------
# basis fit (data-independent; computed once at import)
# ---------------------------------------------------------------------------


def _build_fit(r=R, ridge=1e-6):
    b_r = 4.6 * np.sinh(np.linspace(-1.6, 1.6, r)) / np.sinh(1.6)
    u = np.linspace(-6.5, 6.5, 1301)
    rho = np.exp(-0.5 * u * u) + 0.01
    rho /= rho.sum()
    Phi = np.concatenate(
        [np.ones_like(u)[:, None], u[:, None], np.tanh(u[:, None] + b_r[None, :])],
        axis=1)
    M = Phi.T @ (rho[:, None] * Phi) + ridge * np.eye(r + 2)
    w_grid = np.linspace(-5.0, 5.0, 2001)
    G = np.tanh(u[:, None] + w_grid[None, :])
    C_grid = np.linalg.solve(M, Phi.T @ (rho[:, None] * G))   # [(r+2), Nw]
    return b_r, w_grid, C_grid


_B_R, _W_GRID, _C_GRID = _build_fit()


def _coef_eval(w):
    """Evaluate coefficient functions (rows 1..R+1: linear + R basis) at w."""
    wc = np.clip(w, _W_GRID[0], _W_GRID[-1])
    out = np.empty((R + 1,) + w.shape, dtype=np.float64)
    for i in range(R + 1):
        out[i] = np.interp(wc, _W_GRID, _C_GRID[i + 1])
    return out


def _ceil(x, m):
    return -(-x // m) * m


# ---------------------------------------------------------------------------
# device kernel
# ---------------------------------------------------------------------------


def build_kernel(layout, bound):
    """layout: tuple of segments, each a tuple of slot widths (le_i, padded so
    each segment total is a multiple of 64 -- pad carried by the last slot's
    mask only; slot widths themselves are the DMA/apply sizes)."""
    def chunks_of(n, step=512):
        return [(o, min(step, n - o)) for o in range(0, n, step)]

    segs = [[w for w, _ in s] for s in layout]
    seg_ks = [[k for _, k in s] for s in layout]
    slot_k = [k for s in seg_ks for k in s]
    XH = int(_ceil(max(slot_k), 8))
    seg_T = [sum(s) for s in segs]
    T = sum(seg_T)
    seg_off = np.concatenate([[0], np.cumsum(seg_T)]).astype(int)
    assert all(t % 64 == 0 and t <= SEG_CAP for t in seg_T)
    nseg = len(segs)

    # global slot offsets/widths (in token axis), slot index = (seg, j)
    slot_off = []
    slot_w = []
    for si, s in enumerate(segs):
        o = int(seg_off[si])
        for wdt in s:
            slot_off.append(o)
            slot_w.append(int(wdt))
            o += int(wdt)
    nslots = len(slot_off)
    assert nslots == Q

    nchunks = sum(len(chunks_of(int(seg_off[i + 1]) - int(seg_off[i])))
                  for i in range(nseg))
    CHN = [0]

    nc = bacc.Bacc("TRN2", target_bir_lowering=False, debug=False)

    xt_dram = nc.dram_tensor("xT_in", [128, DC * T], BF16, kind="ExternalInput")
    xp_dram = nc.dram_tensor("x_pad", [XH, Q * D], BF16, kind="ExternalInput")
    m_dram = nc.dram_tensor("m_row_in", [1, T], BF16, kind="ExternalInput")
    uwt_dram = nc.dram_tensor("uwT", [128, DC * A], BF16, kind="ExternalInput")
    co_dram = nc.dram_tensor("coefs", [A, (R + 1) * P], BF16, kind="ExternalInput")
    out_dram = nc.dram_tensor("out", [Q, P, D], BF16, kind="ExternalOutput")
    den_dram = nc.dram_tensor("den", [P, nchunks], F32, kind="ExternalOutput")
    if DEBUG_DUMP:
        uh_dbg = nc.dram_tensor("uh_dbg", [A, T], BF16, kind="ExternalOutput")
        ef_dbg = nc.dram_tensor("ef_dbg", [P, T], BF16, kind="ExternalOutput")

    def chunks(lo, hi, step=512):
        return [(o, min(step, hi - o)) for o in range(lo, hi, step)]

    with TileContext(nc) as tc:
        with tc.tile_pool(name="live", bufs=1) as L:
            ident = L.tile([128, 128], F32)
            ident_b = L.tile([P, P], BF16)
            x_all = L.tile([128, Q * D], BF16)
            xT = L.tile([128, DC * T], BF16)
            uhT = L.tile([A, T], BF16)
            uwt_sb = L.tile([128, DC * A], BF16)
            co_sb = L.tile([A, (R + 1) * P], BF16)
            m_row = L.tile([1, T], BF16)
            ones_b = L.tile([1, P], BF16)
            e_full = L.tile([P, T], BF16)
            aT_all = L.tile([128, Q * P], BF16)
            sumh = L.tile([P, nchunks], F32)
            bvals = L.tile([A, R], F32)
            nbnd = L.tile([P, 1], F32)

            make_identity(nc, ident)
            nc.vector.tensor_copy(ident_b[:], ident[0:P, 0:P])
            nc.gpsimd.memset(ones_b[:], 1.0)
            for r in range(R):
                nc.gpsimd.memset(bvals[:, r:r + 1], float(_B_R[r]))
            nc.gpsimd.memset(nbnd[:], -float(bound))

            # ---- input DMAs (xT first: uh is the critical path) ---------
            # hoist the act-table load: dummy tanh before any DMA lands
            dummy = L.tile([A, 1], BF16)
            nc.scalar.activation(dummy[:], bvals[:, 0:1], AF.Tanh,
                                 bias=0.0, scale=1.0)

            def xt_seg(si):
                s0, s1 = int(seg_off[si]), int(seg_off[si + 1])
                nc.sync.dma_start(
                    xT[:, 0:DC * T].rearrange("r (c t) -> r c t", t=T)[:, :, s0:s1],
                    xt_dram.ap().rearrange("r (c t) -> r c t", t=T)[:, :, s0:s1])

            def xpad_seg(si):
                sl0 = sum(len(s) for s in segs[:si])
                sl1 = sl0 + len(segs[si])
                nc.sync.dma_start(x_all[0:XH, sl0 * D:sl1 * D],
                                  xp_dram.ap()[:, sl0 * D:sl1 * D])

            xt_seg(0)
            nc.sync.dma_start(uwt_sb[:], uwt_dram[:])
            if 'co' not in KSKIP:
                nc.sync.dma_start(co_sb[:], co_dram[:])
            if 'm' not in KSKIP:
                nc.sync.dma_start(m_row[:], m_dram[:])
            for si in range(1, nseg):
                xt_seg(si)
            for si in range(nseg):
                xpad_seg(si)

            with (
                tc.tile_pool(name="scp", bufs=6) as SC,
                tc.tile_pool(name="osp", bufs=3) as OSB,
                tc.tile_pool(name="ups", bufs=1, space="PSUM") as UPS,
                tc.tile_pool(name="pse", bufs=1, space="PSUM") as PE_,
                tc.tile_pool(name="pst", bufs=2, space="PSUM") as PT,
                tc.tile_pool(name="pso", bufs=2, space="PSUM") as PO,
            ):
                # ---- uh for all tokens (segment-aligned chunks) ---------
                uh_chunks = []
                for si in range(nseg):
                    uh_chunks += chunks(int(seg_off[si]), int(seg_off[si + 1]))
                for off, w in uh_chunks:
                    ups = UPS.tile([A, 512], F32, tag="ups")
                    for c in range(DC):
                        nc.tensor.matmul(
                            ups[:, 0:w], uwt_sb[:, c * A:(c + 1) * A],
                            xT[:, c * T + off:c * T + off + w],
                            start=(c == 0), stop=(c == DC - 1))
                    nc.vector.tensor_copy(uhT[:, off:off + w], ups[:, 0:w])

                # ---- per segment: R tanh passes + e matmuls + exp -------
                slot0 = 0
                for si in range(nseg if STAGE >= 1 else 0):
                    s0, s1 = int(seg_off[si]), int(seg_off[si + 1])
                    Th = s1 - s0
                    e_ps = PE_.tile([P, SEG_CAP], F32, tag="eps")
                    for r in range(R):
                        sc = SC.tile([A, SEG_CAP], BF16, tag="sc")
                        nc.scalar.activation(
                            sc[:, 0:Th], uhT[:, s0:s1], AF.Tanh,
                            bias=bvals[:, r:r + 1], scale=1.0)
                        for off, w in (chunks(0, Th) if STAGE >= 2 else []):
                            nc.tensor.matmul(
                                e_ps[:, off:off + w],
                                co_sb[:, (r + 1) * P:(r + 2) * P],
                                sc[:, off:off + w],
                                start=(r == 0), stop=False)
                    seg_slots = list(range(slot0, slot0 + len(segs[si])))
                    slot0 += len(segs[si])
                    for off, w in (chunks(0, Th) if STAGE >= 2 else []):
                        nc.tensor.matmul(
                            e_ps[:, off:off + w], co_sb[:, 0:P],
                            uhT[:, s0 + off:s0 + off + w],
                            start=False, stop=False)
                        nc.tensor.matmul(
                            e_ps[:, off:off + w], ones_b[:, 0:P],
                            m_row[:, s0 + off:s0 + off + w],
                            start=False, stop=True)
                        nc.scalar.activation(
                            e_full[:, s0 + off:s0 + off + w],
                            e_ps[:, off:off + w], AF.Exp,
                            bias=nbnd[:, 0:1], scale=1.0,
                            accum_out=sumh[:, CHN[0]:CHN[0] + 1])
                        CHN[0] += 1
                    groups = [seg_slots[g0:g0 + 3]
                              for g0 in range(0, len(seg_slots), 3)]
                    if STAGE < 3:
                        groups = []

                    def do_group(grp, ops):
                        for j, i in enumerate(grp):
                            le_i = slot_k[i]
                            atp = PT.tile([128, P], BF16, tag="atp")
                            nc.tensor.transpose(
                                atp[0:le_i, :],
                                e_full[:, slot_off[i]:slot_off[i] + le_i],
                                ident_b[:])
                            nc.vector.tensor_copy(
                                aT_all[0:le_i, i * P:(i + 1) * P], atp[0:le_i, :])
                            nc.tensor.matmul(
                                ops[32 * j:32 * (j + 1), :],
                                aT_all[0:le_i, i * P:(i + 1) * P],
                                x_all[0:le_i, i * D:(i + 1) * D],
                                start=True, stop=True)

                    full = [g for g in groups if len(g) == 3]
                    rem = [g for g in groups if len(g) < 3]
                    last_seg = si == nseg - 1
                    bsz = 2 if last_seg else 5
                    for b0 in range(0, len(full), bsz):
                        batch = full[b0:b0 + bsz]
                        nb = len(batch)
                        osb = OSB.tile([128, 5 * D], BF16, tag="osb", bufs=4)
                        for j, grp in enumerate(batch):
                            ops = PO.tile([128, D], F32, tag="ops")
                            do_group(grp, ops)
                            if last_seg:
                                nc.scalar.activation(
                                    osb[0:96, j * D:(j + 1) * D], ops[0:96, :],
                                    AF.Copy, bias=0.0, scale=1.0)
                            else:
                                nc.vector.tensor_copy(
                                    osb[0:96, j * D:(j + 1) * D], ops[0:96, :])
                        s0 = batch[0][0]
                        nc.sync.dma_start(
                            out_dram.ap().rearrange("s p d -> (s p) d")[
                                s0 * P:s0 * P + nb * 96, :].rearrange(
                                    "(g r) d -> r g d", r=96),
                            osb[0:96, 0:nb * D].rearrange("r (g d) -> r g d", d=D))
                    for grp in rem:
                        ops = PO.tile([128, D], F32, tag="ops")
                        do_group(grp, ops)
                        osb2 = OSB.tile([128, D], BF16, tag="osb2", bufs=2)
                        nc.vector.tensor_copy(osb2[0:32 * len(grp), :],
                                              ops[0:32 * len(grp), :])
                        nc.sync.dma_start(
                            out_dram.ap().rearrange("s p d -> (s p) d")[
                                grp[0] * P:(grp[-1] + 1) * P, :],
                            osb2[0:32 * len(grp), :])

            if DEBUG_DUMP:
                nc.sync.dma_start(uh_dbg[:], uhT[:])
                if STAGE >= 2:
                    nc.sync.dma_start(ef_dbg[:], e_full[:])
            if STAGE >= 2:
                nc.sync.dma_start(den_dram[:], sumh[:])
            else:
                nc.gpsimd.memset(sumh[:], 1.0)
                nc.sync.dma_start(den_dram[:], sumh[:])

    nc.compile()
    return nc


_NC_CACHE = {}
LAST_NC = None


def _get_nc(layout, bound):
    key = (layout, round(float(bound), 3))
    if key not in _NC_CACHE:
        _NC_CACHE[key] = build_kernel(layout, bound)
    return _NC_CACHE[key]


# ---------------------------------------------------------------------------
# host entry point
# ---------------------------------------------------------------------------


SEG_FRACS = (0.2, 0.65)  # cumulative split fractions (None = balanced)


def _make_layout(le_prof):
    """Split the descending per-rank profile into segments with padded total
    <= SEG_CAP each, balancing totals; widths padded so each segment total is
    a multiple of 64 (pad added to the last slot of the segment)."""
    tot = int(le_prof.sum())
    pref = np.concatenate([[0], np.cumsum(le_prof)])
    if SEG_FRACS is not None:
        bounds = [0] + [int(np.argmin(np.abs(pref - tot * f)))
                        for f in SEG_FRACS] + [Q]
        nseg = len(SEG_FRACS) + 1
    else:
        nseg = max(2, int(np.ceil(_ceil(tot, 64) / SEG_CAP)))
        bounds = [0]
        for k in range(1, nseg):
            target = tot * k / nseg
            bounds.append(int(np.argmin(np.abs(pref - target))))
        bounds.append(Q)
    def build(bnds):
        segs = []
        for k in range(len(bnds) - 1):
            raw = [int(v) for v in le_prof[bnds[k]:bnds[k + 1]]]
            if not raw:
                return None
            wdts = list(raw)
            wdts[-1] += int(_ceil(sum(wdts), 64) - sum(wdts))
            if sum(wdts) > SEG_CAP:
                return None
            segs.append(tuple(zip(wdts, raw)))
        return tuple(segs)

    segs = build(bounds)
    if segs is None:
        # fallback: balanced split into as many segments as needed
        nseg = max(2, int(np.ceil(_ceil(tot, 64) / (SEG_CAP - 128))))
        bounds = [0] + [int(np.argmin(np.abs(pref - tot * k / nseg)))
                        for k in range(1, nseg)] + [Q]
        segs = build(bounds)
    assert segs is not None
    return segs


def kernel(exp_tokens, exp_mask, s_j, req_mask, Ws_w, Ws_b, U_w, v_w):
    """Full-input entry point: shard over B across 8 cores, gather output."""
    from concourse.bass_utils import run_bass_kernel_spmd

    exp_tokens = np.asarray(exp_tokens, dtype=np.float32)
    exp_mask = np.asarray(exp_mask, dtype=np.int32)
    s_j = np.asarray(s_j, dtype=np.float32)
    req_mask = np.asarray(req_mask, dtype=np.int32)
    Ws_w = np.asarray(Ws_w, dtype=np.float32)
    Ws_b = np.asarray(Ws_b, dtype=np.float32)
    U_w = np.asarray(U_w, dtype=np.float32)
    v_w = np.asarray(v_w, dtype=np.float32)

    # ---- per-rank compaction profile ------------------------------------
    counts = exp_mask.sum(axis=2)                      # [B, Q]
    order = np.argsort(-counts, axis=1, kind="stable")
    sorted_counts = np.take_along_axis(counts, order, axis=1)
    le_prof = sorted_counts.max(axis=0)                # [Q]
    layout = _make_layout(le_prof)
    slot_w = [w for s in layout for w, _ in s]
    slot_k = [k for s in layout for _, k in s]
    XH = int(_ceil(max(slot_k), 8))
    slot_off = np.concatenate([[0], np.cumsum(slot_w)]).astype(int)
    T = int(slot_off[-1])

    # ---- compacted x + mask row ----------------------------------------
    x_c = np.zeros((B, T, D), dtype=np.float32)
    m_row = np.full((B, 1, T), -1e9, dtype=np.float32)
    for b in range(B):
        for i in range(Q):
            qo = order[b, i]
            idx = np.flatnonzero(exp_mask[b, qo])
            n = len(idx)
            o = slot_off[i]
            x_c[b, o:o + n] = exp_tokens[b, qo, idx]
            m_row[b, 0, o:o + n] = 0.0

    # ---- host coefficients ---------------------------------------------
    ws = np.einsum("bpd,ad->bpa", s_j, Ws_w, optimize=True) + Ws_b  # [B,P,A]
    co = _coef_eval(ws) * v_w[0][None, None, None, :]  # [(R+1), B, P, A]
    # stationary layout [A, (R+1)*P], order: linear first, then basis r
    coefs = np.ascontiguousarray(
        np.transpose(co, (1, 3, 0, 2)).reshape(B, A, (R + 1) * P))
    bound = float(np.abs(co[1:]).sum(axis=(0, 3)).max()
                  + 6.0 * np.abs(co[0]).sum(axis=2).max()) + 1.0
    bound = _ceil(bound, 4.0)

    # uwT: [128, DC*A] with uwT[dd, c*A+a] = U_w[a, c*128+dd]
    uwT = np.ascontiguousarray(
        U_w.reshape(A, DC, 128).transpose(2, 1, 0).reshape(128, DC * A))

    nc = _get_nc(layout, bound)
    global LAST_NC
    LAST_NC = nc

    xT_host = np.ascontiguousarray(
        x_c.reshape(B, T, DC, 128).transpose(0, 3, 2, 1)).reshape(B, 128, DC * T)
    xT_bf = xT_host.astype(ml_dtypes.bfloat16)
    x_pad = np.zeros((B, XH, Q, D), dtype=np.float32)
    for i in range(Q):
        o, k = int(slot_off[i]), slot_k[i]
        x_pad[:, 0:k, i, :] = x_c[:, o:o + k, :]
    x_pad_bf = x_pad.reshape(B, XH, Q * D).astype(ml_dtypes.bfloat16)
    uwT_bf = uwT.astype(ml_dtypes.bfloat16)
    coefs_bf = coefs.astype(ml_dtypes.bfloat16)
    in_maps = []
    for b in range(B):
        in_maps.append({
            "xT_in": xT_bf[b],
            "x_pad": x_pad_bf[b],
            "m_row_in": m_row[b].astype(ml_dtypes.bfloat16),
            "uwT": uwT_bf,
            "coefs": coefs_bf[b],
        })
    res = run_bass_kernel_spmd(nc, in_maps, core_ids=list(range(N_CORES)))

    out = np.empty((B, Q, P, D), dtype=np.float32)
    for b in range(B):
        o_slot = res.results[b]["out"].astype(np.float32)   # [Q, P, D]
        den = res.results[b]["den"].astype(np.float64).sum(axis=1)  # [P]
        scale = (req_mask[b].astype(np.float64) / (den + 1e-300)).astype(np.float32)
        o_slot *= scale[None, :, None]
        out[b, order[b]] = o_slot
    return out
